# revision 19
# baseline (speedup 1.0000x reference)
"""AttentionPerLabelWordLevel Trainium2 kernel (8-core SPMD, batch-sharded).

Reference computation (per batch b):
  h = tanh(x @ W.T + b)                      # [T, H]
  logits = h @ C.T                           # [S, L, C]
  m = max_L(logits)                          # [S, 1, C]
  attn = softmax_C(logits - m)               # [S, L, C]
  out[s, c, :] = sum_l attn[s, l, c] * x[s, l, :]   # [S, C, H]

Shapes: B=32, T=2500 (S=100 sentences x L=25 words), H=512, C=50.
Sharding: data-parallel over batch, 4 batches per core.

Final design (HW: ~272 us/core vs 430 us baseline):
  - Host-side prep (free): x^T pretransposed ([B_loc, H, T] f16), x
    natural pre-packed into the SBUF einsum image (4 sentences per
    128-partition tile at 32-row offsets), W^T / C^T pre-arranged.
  - step 1/2 run on a DENSE t axis (no 25->32 padding): 6 waves per
    batch (5x16 + 1x20 sentences), dense f16 matmuls N=400/500.
  - f16 output stored in an SBUF-mirror layout with 8-12 KB contiguous
    DMA lines, decoded on the host; A half (partitions 0-49) on the
    GpSimd SWDGE ring, B half (64-113) on the Sync HWDGE ring so the
    two halves drain through complementary SDMA engines concurrently.
  - 5-stage software pipeline (lags: logits 2, softmax 3, e^T 4,
    einsum 5 iterations behind step1) so every engine-FIFO head has
    its producers >=1 iteration old; einsum matmuls are split around
    the step1 block to hide the PSUM-ring handoff.
  - PSUM: ph ring x4 banks (breaks the step1<->tanh lockstep),
    pl x1, shared {po, pet} ring x3 = 8 banks.
"""

import numpy as np

import concourse.bacc as bacc
import concourse.bass as bass
import concourse.tile as tile
from concourse import mybir
from concourse.bass_utils import run_bass_kernel_spmd
from concourse.masks import make_identity

F32 = mybir.dt.float32
F16 = mybir.dt.float16
AX = mybir.AxisListType
AF = mybir.ActivationFunctionType

N_CORES = 8
B = 32
S = 100          # sentences per batch
L = 25           # words per sentence
C = 50           # classes
H = 512          # hidden
B_LOC = B // N_CORES          # batches per core
WAVE_S = 16                   # sentences per full wave
N_WAVES = 6                   # 5 waves of 16 + 1 wave of 20 sentences

_CACHE = {}
LAST_RESULT = None


def build_nc():
    nc = bacc.Bacc(trn_type="TRN2", target_bir_lowering=False, debug=False,
                   num_swdge_queues=2)
    xt_d = nc.declare_dram_parameter("xT", [B_LOC, H, S * L], F16, isOutput=False)
    xp_d = nc.declare_dram_parameter("xp", [B_LOC, N_WAVES, 128, 2560], F16, isOutput=False)
    wt_d = nc.declare_dram_parameter("WT", [H, H], F16, isOutput=False)
    b_d = nc.declare_dram_parameter("b", [H], F32, isOutput=False)
    ct_d = nc.declare_dram_parameter("CT", [128, 4 * C], F16, isOutput=False)
    o_d = nc.declare_dram_parameter("out", [B_LOC, N_WAVES, 2, C, 6144], F16, isOutput=True)

    with tile.TileContext(nc) as tc:
        with tc.tile_pool(name="sb", bufs=1) as sb, \
             tc.tile_pool(name="consts", bufs=1) as consts, \
             tc.tile_pool(name="ps", bufs=1, space="PSUM") as ps:

            # ---------------- one-time setup ----------------
            ident_f = consts.tile([128, 128], F32)
            make_identity(nc, ident_f)
            ident_h = consts.tile([128, 128], F16)
            nc.vector.tensor_copy(ident_h, ident_f)

            b_sb = consts.tile([128, 4], F32)
            nc.sync.dma_start(out=b_sb, in_=b_d.rearrange("(k p) -> p k", p=128))

            w_t = []
            for i in range(4):
                wt = consts.tile([128, 512], F16, name=f"w_t{i}")
                nc.sync.dma_start(out=wt, in_=wt_d[i * 128:(i + 1) * 128, :])
                w_t.append(wt)

            c_t = consts.tile([128, 4 * C], F16)
            nc.sync.dma_start(out=c_t, in_=ct_d[:, :])

            def load_xt(bi):
                tiles = []
                for i in range(4):
                    xt = sb.tile([128, S * L], F16, tag="xt", bufs=8,
                                 name=f"xt{bi}_{i}")
                    nc.sync.dma_start(out=xt, in_=xt_d[bi, i * 128:(i + 1) * 128, :])
                    tiles.append(xt)
                return tiles

            xt_cur = [None, None]
            xt_cur[0] = load_xt(0)

            waves = [(bi, wv) for bi in range(B_LOC) for wv in range(N_WAVES)]
            NW = len(waves)

            state = {}   # wave index -> per-stage tiles

            def wave_dims(wi):
                bi, wv = waves[wi]
                ns = WAVE_S if wv < 5 else 20
                return bi, wv, ns, ns // 4, ns * L

            def emit_load(wi):
                bi, wv, ns, G, TCOL = wave_dims(wi)
                xp_t = sb.tile([128, 2560], F16, tag="xp", bufs=7,
                               name=f"xp{bi}_{wv}")
                for jj in range(4):
                    nc.sync.dma_start(
                        out=xp_t[32 * jj:32 * jj + L, :512 * G],
                        in_=xp_d[bi, wv, 32 * jj:32 * jj + L, :512 * G])
                state[wi] = {"xp": xp_t}

            def emit_F(wi):
                bi, wv, ns, G, TCOL = wave_dims(wi)
                xts = xt_cur[bi % 2]
                t0 = wv * WAVE_S * L
                st = state[wi]

                # step 1: h^T[o] = tanh(W @ x^T + b), dense t
                h = []
                for o in range(4):
                    ph = ps.tile([128, 512], F32, tag="ph", bufs=4,
                                 name=f"ph{bi}_{wv}_{o}")
                    for i in range(4):
                        nc.tensor.matmul(
                            ph[:, :TCOL],
                            w_t[i][:, o * 128:(o + 1) * 128],
                            xts[i][:, t0:t0 + TCOL],
                            start=(i == 0), stop=(i == 3),
                        )
                    ht = sb.tile([128, 512], F16, tag="h", bufs=12,
                                 name=f"h{bi}_{wv}_{o}")
                    nc.scalar.activation(
                        out=ht[:, :TCOL], in_=ph[:, :TCOL],
                        func=AF.Tanh, bias=b_sb[:, o:o + 1], scale=1.0,
                    )
                    h.append(ht)
                st["h"] = h

            def emit_L(wi):
                # step 2: logits[c, t] dense (accumulate over o) - deferred
                # one iteration so tanh(w) has finished on ScalarE
                bi, wv, ns, G, TCOL = wave_dims(wi)
                st = state[wi]
                h = st.pop("h")
                pl = ps.tile([64, 512], F32, tag="pl", bufs=1,
                             name=f"pl{bi}_{wv}")
                for o in range(4):
                    nc.tensor.matmul(
                        pl[:C, :TCOL], c_t[:, o * C:(o + 1) * C],
                        h[o][:, :TCOL],
                        start=(o == 0), stop=(o == 3),
                    )
                st["pl"] = pl

            def emit_A(wi):
                bi, wv, ns, G, TCOL = wave_dims(wi)
                st = state[wi]
                pl = st["pl"]

                plb = pl[:C, :]
                pl_v = bass.AP(tensor=pl.tensor, offset=plb.offset,
                               ap=[plb.ap[0], [L, ns], [1, L]])
                m = sb.tile([64, 20], F32, tag="m", bufs=4,
                            name=f"m{bi}_{wv}")
                nc.vector.reduce_max(out=m[:C, :ns], in_=pl_v, axis=AX.X)

                epre = sb.tile([64, 640], F16, tag="epre", bufs=4,
                               name=f"epre{bi}_{wv}")
                e_sb = sb.tile([64, 640], F16, tag="e", bufs=4,
                               name=f"e{bi}_{wv}")
                eb = epre[:C, :]
                ep_v = bass.AP(tensor=epre.tensor, offset=eb.offset,
                               ap=[eb.ap[0], [32, ns], [1, L]])
                esb = e_sb[:C, :]
                e_v = bass.AP(tensor=e_sb.tensor, offset=esb.offset,
                              ap=[esb.ap[0], [32, ns], [1, L]])
                mb = m[:C, :]
                m_v = bass.AP(tensor=m.tensor, offset=mb.offset,
                              ap=[mb.ap[0], [1, ns], [0, L]])
                nc.vector.tensor_sub(ep_v, pl_v, m_v)
                nc.scalar.activation(out=e_v, in_=ep_v, func=AF.Exp)
                st["e"] = e_sb

            def emit_B(wi):
                bi, wv, ns, G, TCOL = wave_dims(wi)
                st = state[wi]
                e_sb = st["e"]

                pet = ps.tile([128, 320], F16, tag="pp", bufs=3,
                              name=f"pet{bi}_{wv}")
                for g in range(G):
                    nc.tensor.transpose(
                        pet[:, 64 * g:64 * g + C],
                        e_sb[:C, 128 * g:128 * (g + 1)],
                        ident_h[:C, :C],
                    )
                attn = sb.tile([128, 320], F16, tag="attn", bufs=4,
                               name=f"attn{bi}_{wv}")
                nc.vector.tensor_copy(attn.bitcast(F32), pet.bitcast(F32))

                att_v = bass.AP(tensor=attn.tensor, offset=attn.offset,
                                ap=[attn.ap[0], [64, G], [1, C]])
                z = sb.tile([128, 8], F32, tag="z", bufs=3,
                            name=f"z{bi}_{wv}")
                nc.vector.reduce_sum(out=z[:, :G], in_=att_v, axis=AX.X)
                nc.vector.reciprocal(out=z[:, :G], in_=z[:, :G])
                z_v = bass.AP(tensor=z.tensor, offset=z.offset,
                              ap=[z.ap[0], [1, G], [0, C]])
                nc.gpsimd.tensor_mul(att_v, att_v, z_v)
                st["attn"] = attn

            def emit_dummies(wi_pl, n):
                if wi_pl < 0 or wi_pl not in state:
                    return
                pl = state[wi_pl].get("pl")
                if pl is None:
                    return
                for k in range(n):
                    nc.tensor.matmul(
                        pl[0:32, :128], ident_h[:, :32],
                        w_t[k % 4][:, :128], start=True, stop=True)

            def emit_C_mms(wi, po_range):
                """Einsum matmuls + psum->sbuf copies for po indices in
                po_range (a po = one (pair, jj): up to 2 tile-packed MMs)."""
                bi, wv, ns, G, TCOL = wave_dims(wi)
                st = state[wi]
                attn = st["attn"]
                xp_t = st["xp"]
                npo = 4 * ((G + 1) // 2)
                if "osb" not in st:
                    st["osb"] = sb.tile([128, 6144], F16, tag="osb", bufs=3,
                                        name=f"osb{bi}_{wv}")
                osb = st["osb"]
                for k in po_range:
                    if k >= npo:
                        continue
                    pi, jj = k // 4, k % 4
                    gl_count = 2 if 2 * pi + 1 < G else 1
                    nrow = 64 * (gl_count - 1) + C
                    po = ps.tile([128, 512], F32, tag="pp", bufs=3,
                                 name=f"po{bi}_{wv}_{pi}_{jj}")
                    for gl in range(gl_count):
                        g = 2 * pi + gl
                        nc.tensor.matmul(
                            po[64 * gl:64 * gl + C, :],
                            attn[32 * jj:32 * jj + L, 64 * g:64 * g + C],
                            xp_t[32 * jj:32 * jj + L, 512 * g:512 * (g + 1)],
                            start=True, stop=True,
                            tile_position=(32 * jj, 64 * gl),
                        )
                    dstc = osb[:nrow, 512 * k:512 * (k + 1)]
                    if k % 4 == 3:
                        nc.scalar.copy(dstc, po[:nrow, :])
                    else:
                        nc.vector.tensor_copy(dstc, po[:nrow, :])

            def emit_store(wi):
                bi, wv, ns, G, TCOL = wave_dims(wi)
                st = state[wi]
                osb = st["osb"]
                ncols_a = 512 * 4 * ((G + 1) // 2)
                ncols_b = 2048 * (G // 2)
                nc.gpsimd.dma_start(
                    out=o_d[bi, wv, 0, :, :ncols_a], in_=osb[:C, :ncols_a])
                nc.sync.dma_start(
                    out=o_d[bi, wv, 1, :, :ncols_b],
                    in_=osb[64:64 + C, :ncols_b])
                del state[wi]

            # ---------------- pipelined emission ----------------
            # iteration i: A(i-3), C(i-5) split around F(i), L(i-2), B(i-4)
            LG_L, LG_A, LG_B, LG_C = 2, 3, 4, 5
            for i in range(NW + LG_C):
                if i < NW:
                    bi, wv = waves[i]
                    if wv == 4 and bi + 1 < B_LOC:
                        xt_cur[(bi + 1) % 2] = load_xt(bi + 1)
                    if i == 0:
                        emit_load(0)
                if i - LG_A >= 0 and i - LG_A < NW:
                    emit_A(i - LG_A)
                    emit_dummies(i - LG_A, 4)
                if i - LG_C >= 0:
                    emit_C_mms(i - LG_C, range(0, 3))
                    emit_dummies(i - LG_A, 2)
                    emit_C_mms(i - LG_C, range(3, 6))
                if i < NW:
                    emit_F(i)
                    if i + 1 < NW:
                        emit_load(i + 1)
                if i - LG_C >= 0:
                    emit_C_mms(i - LG_C, range(6, 9))
                    emit_dummies(i - LG_A, 2)
                    emit_C_mms(i - LG_C, range(9, 12))
                    emit_store(i - LG_C)
                if i - LG_L >= 0 and i - LG_L < NW:
                    emit_dummies(i - LG_A, 6)
                    emit_L(i - LG_L)
                if i - LG_B >= 0 and i - LG_B < NW:
                    emit_B(i - LG_B)

    nc.compile()
    return nc


def _host_prep(x16):
    """Build per-core xT (host-transposed) and xp (packed SBUF image)."""
    xT = np.ascontiguousarray(x16.transpose(0, 2, 1))           # [B_LOC, H, T]
    xs = x16.reshape(B_LOC, S, L, H)
    xp = np.zeros((B_LOC, N_WAVES, 128, 2560), dtype=np.float16)
    for g in range(4):
        for jj in range(4):
            sidx = np.arange(5) * WAVE_S + 4 * g + jj
            xp[:, :5, 32 * jj:32 * jj + L, 512 * g:512 * (g + 1)] = \
                xs[:, sidx, :, :]
    for g in range(5):
        for jj in range(4):
            xp[:, 5, 32 * jj:32 * jj + L, 512 * g:512 * (g + 1)] = \
                xs[:, 80 + 4 * g + jj, :, :]
    return xT, xp


def _host_decode(raw):
    """Decode the SBUF-mirror f16 output into [B_LOC, S, C, H] f32."""
    out = np.empty((B_LOC, S, C, H), dtype=np.float32)
    v = raw.reshape(B_LOC, N_WAVES, 2, C, 12, H)
    for wv in range(6):
        for k in range(8):
            s0 = WAVE_S * wv + 8 * (k // 4)
            out[:, s0 + (k % 4)] = v[:, wv, 0, :, k]
            out[:, s0 + 4 + (k % 4)] = v[:, wv, 1, :, k]
    for k in range(8, 12):
        out[:, 96 + (k % 4)] = v[:, 5, 0, :, k]
    return out


def kernel(**inputs):
    global LAST_RESULT
    if "nc" not in _CACHE:
        _CACHE["nc"] = build_nc()
    nc = _CACHE["nc"]

    x = np.asarray(inputs["input_tensor"], dtype=np.float32).astype(np.float16)
    w = np.asarray(inputs["W"], dtype=np.float32).astype(np.float16)
    bb = np.ascontiguousarray(inputs["b"], dtype=np.float32)
    cv = np.asarray(inputs["context_vector"], dtype=np.float32).astype(np.float16)

    wt_h = np.ascontiguousarray(w.T)                            # [I, O]
    ct_h = np.ascontiguousarray(
        cv.T.reshape(4, 128, C).transpose(1, 0, 2).reshape(128, 4 * C))

    in_maps = []
    for ci in range(N_CORES):
        xT, xp = _host_prep(x[ci * B_LOC:(ci + 1) * B_LOC])
        in_maps.append({
            "xT": xT,
            "xp": xp,
            "WT": wt_h,
            "b": bb,
            "CT": ct_h,
        })
    res = run_bass_kernel_spmd(nc, in_maps, core_ids=list(range(N_CORES)))
    LAST_RESULT = res
    out = np.empty((B, S, C, H), dtype=np.float32)
    for ci in range(N_CORES):
        out[ci * B_LOC:(ci + 1) * B_LOC] = _host_decode(res.results[ci]["out"])
    return out


# revision 23
# speedup vs baseline: 1.2852x; 1.2852x over previous
"""AttentionPerLabelWordLevel Trainium2 kernel (8-core SPMD, batch-sharded).

Reference computation (per batch b):
  h = tanh(x @ W.T + b)                      # [T, H]
  logits = h @ C.T                           # [S, L, C]
  m = max_L(logits)                          # [S, 1, C]
  attn = softmax_C(logits - m)               # [S, L, C]
  out[s, c, :] = sum_l attn[s, l, c] * x[s, l, :]   # [S, C, H]

Shapes: B=32, T=2500 (S=100 sentences x L=25 words), H=512, C=50.
Sharding: data-parallel over batch, 4 batches per core.

Final design (HW: ~235 us/core vs 430 us baseline):
  - Host-side prep (free): x^T pretransposed ([B_loc, H, T] f16), x
    natural pre-packed into the SBUF einsum image (4 sentences per
    128-partition tile at 32-row offsets), W^T / C^T pre-arranged.
  - step 1/2 run on a DENSE t axis (no 25->32 padding): 6 waves per
    batch (5x16 + 1x20 sentences), dense f16 matmuls N=400/500.
  - f16 output stored in an SBUF-mirror layout with 8-12 KB contiguous
    DMA lines, decoded on the host; A half (partitions 0-49) on the
    GpSimd SWDGE ring, B half (64-113) on the Sync HWDGE ring so the
    two halves drain through complementary SDMA engines concurrently.
  - 5-stage software pipeline (lags: logits 2, softmax 3, e^T 4,
    einsum 5 iterations behind step1) so every engine-FIFO head has
    its producers >=1 iteration old; einsum matmuls are split around
    the step1 block to hide the PSUM-ring handoff. The e^T/attn-chain
    stage is emitted at the HEAD of each iteration (its consumers run
    at the start of the next one); einsum PSUM->SBUF copies alternate
    DVE/ACT 50/50.
  - PSUM: ph ring x4 banks (breaks the step1<->tanh lockstep),
    pl x1, shared {po, pet} ring x3 = 8 banks.
"""

import numpy as np

import concourse.bacc as bacc
import concourse.bass as bass
import concourse.tile as tile
from concourse import mybir
from concourse.bass_utils import run_bass_kernel_spmd
from concourse.masks import make_identity

F32 = mybir.dt.float32
F16 = mybir.dt.float16
AX = mybir.AxisListType
AF = mybir.ActivationFunctionType

N_CORES = 8
B = 32
S = 100          # sentences per batch
L = 25           # words per sentence
C = 50           # classes
H = 512          # hidden
B_LOC = B // N_CORES          # batches per core
WAVE_S = 16                   # sentences per full wave
N_WAVES = 6                   # 5 waves of 16 + 1 wave of 20 sentences

_CACHE = {}
LAST_RESULT = None


def build_nc():
    nc = bacc.Bacc(trn_type="TRN2", target_bir_lowering=False, debug=False,
                   num_swdge_queues=2)
    xt_d = nc.declare_dram_parameter("xT", [B_LOC, H, S * L], F16, isOutput=False)
    xp_d = nc.declare_dram_parameter("xp", [B_LOC, N_WAVES, 128, 2560], F16, isOutput=False)
    wt_d = nc.declare_dram_parameter("WT", [H, H], F16, isOutput=False)
    b_d = nc.declare_dram_parameter("b", [H], F32, isOutput=False)
    ct_d = nc.declare_dram_parameter("CT", [128, 4 * C], F16, isOutput=False)
    o_d = nc.declare_dram_parameter("out", [B_LOC, N_WAVES, 2, C, 6144], F16, isOutput=True)

    with tile.TileContext(nc) as tc:
        with tc.tile_pool(name="sb", bufs=1) as sb, \
             tc.tile_pool(name="consts", bufs=1) as consts, \
             tc.tile_pool(name="ps", bufs=1, space="PSUM") as ps:

            # ---------------- one-time setup ----------------
            ident_f = consts.tile([128, 128], F32)
            make_identity(nc, ident_f)
            ident_h = consts.tile([128, 128], F16)
            nc.vector.tensor_copy(ident_h, ident_f)

            b_sb = consts.tile([128, 4], F32)
            nc.sync.dma_start(out=b_sb, in_=b_d.rearrange("(k p) -> p k", p=128))

            w_t = []
            for i in range(4):
                wt = consts.tile([128, 512], F16, name=f"w_t{i}")
                nc.sync.dma_start(out=wt, in_=wt_d[i * 128:(i + 1) * 128, :])
                w_t.append(wt)

            c_t = consts.tile([128, 4 * C], F16)
            nc.sync.dma_start(out=c_t, in_=ct_d[:, :])

            def load_xt(bi):
                tiles = []
                for i in range(4):
                    xt = sb.tile([128, S * L], F16, tag="xt", bufs=8,
                                 name=f"xt{bi}_{i}")
                    nc.gpsimd.dma_start(out=xt, in_=xt_d[bi, i * 128:(i + 1) * 128, :])
                    tiles.append(xt)
                return tiles

            xt_cur = [None, None]
            xt_cur[0] = load_xt(0)

            waves = [(bi, wv) for bi in range(B_LOC) for wv in range(N_WAVES)]
            NW = len(waves)

            state = {}   # wave index -> per-stage tiles

            def wave_dims(wi):
                bi, wv = waves[wi]
                ns = WAVE_S if wv < 5 else 20
                return bi, wv, ns, ns // 4, ns * L

            def emit_load(wi):
                bi, wv, ns, G, TCOL = wave_dims(wi)
                xp_t = sb.tile([128, 2560], F16, tag="xp", bufs=7,
                               name=f"xp{bi}_{wv}")
                for jj in range(4):
                    nc.sync.dma_start(
                        out=xp_t[32 * jj:32 * jj + L, :512 * G],
                        in_=xp_d[bi, wv, 32 * jj:32 * jj + L, :512 * G])
                state[wi] = {"xp": xp_t}

            def emit_F(wi):
                bi, wv, ns, G, TCOL = wave_dims(wi)
                xts = xt_cur[bi % 2]
                t0 = wv * WAVE_S * L
                st = state[wi]

                # step 1: h^T[o] = tanh(W @ x^T + b), dense t
                h = []
                for o in range(4):
                    ph = ps.tile([128, 512], F32, tag="ph", bufs=4,
                                 name=f"ph{bi}_{wv}_{o}")
                    for i in range(4):
                        nc.tensor.matmul(
                            ph[:, :TCOL],
                            w_t[i][:, o * 128:(o + 1) * 128],
                            xts[i][:, t0:t0 + TCOL],
                            start=(i == 0), stop=(i == 3),
                        )
                    ht = sb.tile([128, 512], F16, tag="h", bufs=12,
                                 name=f"h{bi}_{wv}_{o}")
                    nc.scalar.activation(
                        out=ht[:, :TCOL], in_=ph[:, :TCOL],
                        func=AF.Tanh, bias=b_sb[:, o:o + 1], scale=1.0,
                    )
                    h.append(ht)
                st["h"] = h

            def emit_L(wi):
                # step 2: logits[c, t] dense (accumulate over o) - deferred
                # one iteration so tanh(w) has finished on ScalarE
                bi, wv, ns, G, TCOL = wave_dims(wi)
                st = state[wi]
                h = st.pop("h")
                pl = ps.tile([64, 512], F32, tag="pl", bufs=1,
                             name=f"pl{bi}_{wv}")
                for o in range(4):
                    nc.tensor.matmul(
                        pl[:C, :TCOL], c_t[:, o * C:(o + 1) * C],
                        h[o][:, :TCOL],
                        start=(o == 0), stop=(o == 3),
                    )
                st["pl"] = pl

            def emit_A(wi):
                bi, wv, ns, G, TCOL = wave_dims(wi)
                st = state[wi]
                pl = st["pl"]

                plb = pl[:C, :]
                pl_v = bass.AP(tensor=pl.tensor, offset=plb.offset,
                               ap=[plb.ap[0], [L, ns], [1, L]])
                m = sb.tile([64, 20], F32, tag="m", bufs=4,
                            name=f"m{bi}_{wv}")
                nc.vector.reduce_max(out=m[:C, :ns], in_=pl_v, axis=AX.X)

                epre = sb.tile([64, 640], F16, tag="epre", bufs=4,
                               name=f"epre{bi}_{wv}")
                e_sb = sb.tile([64, 640], F16, tag="e", bufs=4,
                               name=f"e{bi}_{wv}")
                eb = epre[:C, :]
                ep_v = bass.AP(tensor=epre.tensor, offset=eb.offset,
                               ap=[eb.ap[0], [32, ns], [1, L]])
                esb = e_sb[:C, :]
                e_v = bass.AP(tensor=e_sb.tensor, offset=esb.offset,
                              ap=[esb.ap[0], [32, ns], [1, L]])
                mb = m[:C, :]
                m_v = bass.AP(tensor=m.tensor, offset=mb.offset,
                              ap=[mb.ap[0], [1, ns], [0, L]])
                nc.vector.tensor_sub(ep_v, pl_v, m_v)
                nc.scalar.activation(out=e_v, in_=ep_v, func=AF.Exp)
                st["e"] = e_sb

            def emit_B(wi):
                bi, wv, ns, G, TCOL = wave_dims(wi)
                st = state[wi]
                e_sb = st["e"]

                pet = ps.tile([128, 320], F16, tag="pp", bufs=3,
                              name=f"pet{bi}_{wv}")
                for g in range(G):
                    nc.tensor.transpose(
                        pet[:, 64 * g:64 * g + C],
                        e_sb[:C, 128 * g:128 * (g + 1)],
                        ident_h[:C, :C],
                    )
                attn = sb.tile([128, 320], F16, tag="attn", bufs=4,
                               name=f"attn{bi}_{wv}")
                nc.vector.tensor_copy(attn.bitcast(F32), pet.bitcast(F32))

                att_v = bass.AP(tensor=attn.tensor, offset=attn.offset,
                                ap=[attn.ap[0], [64, G], [1, C]])
                z = sb.tile([128, 8], F32, tag="z", bufs=3,
                            name=f"z{bi}_{wv}")
                nc.vector.reduce_sum(out=z[:, :G], in_=att_v, axis=AX.X)
                nc.vector.reciprocal(out=z[:, :G], in_=z[:, :G])
                z_v = bass.AP(tensor=z.tensor, offset=z.offset,
                              ap=[z.ap[0], [1, G], [0, C]])
                nc.vector.tensor_mul(att_v, att_v, z_v)
                st["attn"] = attn

            def emit_dummies(wi_pl, n):
                if wi_pl < 0 or wi_pl not in state:
                    return
                pl = state[wi_pl].get("pl")
                if pl is None:
                    return
                for k in range(n):
                    nc.tensor.matmul(
                        pl[0:32, :128], ident_h[:, :32],
                        w_t[k % 4][:, :128], start=True, stop=True)

            def emit_C_mms(wi, po_range):
                """Einsum matmuls + psum->sbuf copies for po indices in
                po_range (a po = one (pair, jj): up to 2 tile-packed MMs)."""
                bi, wv, ns, G, TCOL = wave_dims(wi)
                st = state[wi]
                attn = st["attn"]
                xp_t = st["xp"]
                npo = 4 * ((G + 1) // 2)
                if "osb" not in st:
                    st["osb"] = sb.tile([128, 6144], F16, tag="osb", bufs=3,
                                        name=f"osb{bi}_{wv}")
                osb = st["osb"]
                for k in po_range:
                    if k >= npo:
                        continue
                    pi, jj = k // 4, k % 4
                    gl_count = 2 if 2 * pi + 1 < G else 1
                    nrow = 64 * (gl_count - 1) + C
                    po = ps.tile([128, 512], F32, tag="pp", bufs=3,
                                 name=f"po{bi}_{wv}_{pi}_{jj}")
                    for gl in range(gl_count):
                        g = 2 * pi + gl
                        nc.tensor.matmul(
                            po[64 * gl:64 * gl + C, :],
                            attn[32 * jj:32 * jj + L, 64 * g:64 * g + C],
                            xp_t[32 * jj:32 * jj + L, 512 * g:512 * (g + 1)],
                            start=True, stop=True,
                            tile_position=(32 * jj, 64 * gl),
                        )
                    dstc = osb[:nrow, 512 * k:512 * (k + 1)]
                    if k % 2 == 1:
                        nc.scalar.copy(dstc, po[:nrow, :])
                    else:
                        nc.vector.tensor_copy(dstc, po[:nrow, :])

            def emit_store(wi):
                bi, wv, ns, G, TCOL = wave_dims(wi)
                st = state[wi]
                osb = st["osb"]
                ncols_a = 512 * 4 * ((G + 1) // 2)
                ncols_b = 2048 * (G // 2)
                nc.gpsimd.dma_start(
                    out=o_d[bi, wv, 0, :, :ncols_a], in_=osb[:C, :ncols_a])
                nc.sync.dma_start(
                    out=o_d[bi, wv, 1, :, :ncols_b],
                    in_=osb[64:64 + C, :ncols_b])
                del state[wi]

            # ---------------- pipelined emission ----------------
            # iteration i: A(i-3), C(i-5) split around F(i), L(i-2), B(i-4)
            LG_L, LG_A, LG_B, LG_C = 2, 3, 4, 5
            for i in range(NW + LG_C):
                if i < NW:
                    bi, wv = waves[i]
                    if wv == 4 and bi + 1 < B_LOC:
                        xt_cur[(bi + 1) % 2] = load_xt(bi + 1)
                    if i == 0:
                        emit_load(0)
                if i - LG_A >= 0 and i - LG_A < NW:
                    emit_A(i - LG_A)
                    emit_dummies(i - LG_A, 4)
                if i - LG_C >= 0:
                    emit_C_mms(i - LG_C, range(0, 3))
                    emit_dummies(i - LG_A, 2)
                    emit_C_mms(i - LG_C, range(3, 6))
                if i < NW:
                    emit_F(i)
                    if i + 1 < NW:
                        emit_load(i + 1)
                if i - LG_C >= 0:
                    emit_C_mms(i - LG_C, range(6, 9))
                    emit_dummies(i - LG_A, 2)
                    emit_C_mms(i - LG_C, range(9, 12))
                    emit_store(i - LG_C)
                if i - LG_L >= 0 and i - LG_L < NW:
                    emit_dummies(i - LG_A, 6)
                    emit_L(i - LG_L)
                if i - LG_B >= 0 and i - LG_B < NW:
                    emit_B(i - LG_B)

    nc.compile()
    return nc


def _host_prep(x16):
    """Build per-core xT (host-transposed) and xp (packed SBUF image)."""
    xT = np.ascontiguousarray(x16.transpose(0, 2, 1))           # [B_LOC, H, T]
    xs = x16.reshape(B_LOC, S, L, H)
    xp = np.zeros((B_LOC, N_WAVES, 128, 2560), dtype=np.float16)
    for g in range(4):
        for jj in range(4):
            sidx = np.arange(5) * WAVE_S + 4 * g + jj
            xp[:, :5, 32 * jj:32 * jj + L, 512 * g:512 * (g + 1)] = \
                xs[:, sidx, :, :]
    for g in range(5):
        for jj in range(4):
            xp[:, 5, 32 * jj:32 * jj + L, 512 * g:512 * (g + 1)] = \
                xs[:, 80 + 4 * g + jj, :, :]
    return xT, xp


def _host_decode(raw):
    """Decode the SBUF-mirror f16 output into [B_LOC, S, C, H] f32."""
    out = np.empty((B_LOC, S, C, H), dtype=np.float32)
    v = raw.reshape(B_LOC, N_WAVES, 2, C, 12, H)
    for wv in range(6):
        for k in range(8):
            s0 = WAVE_S * wv + 8 * (k // 4)
            out[:, s0 + (k % 4)] = v[:, wv, 0, :, k]
            out[:, s0 + 4 + (k % 4)] = v[:, wv, 1, :, k]
    for k in range(8, 12):
        out[:, 96 + (k % 4)] = v[:, 5, 0, :, k]
    return out


def kernel(**inputs):
    global LAST_RESULT
    if "nc" not in _CACHE:
        _CACHE["nc"] = build_nc()
    nc = _CACHE["nc"]

    x = np.asarray(inputs["input_tensor"], dtype=np.float32).astype(np.float16)
    w = np.asarray(inputs["W"], dtype=np.float32).astype(np.float16)
    bb = np.ascontiguousarray(inputs["b"], dtype=np.float32)
    cv = np.asarray(inputs["context_vector"], dtype=np.float32).astype(np.float16)

    wt_h = np.ascontiguousarray(w.T)                            # [I, O]
    ct_h = np.ascontiguousarray(
        cv.T.reshape(4, 128, C).transpose(1, 0, 2).reshape(128, 4 * C))

    in_maps = []
    for ci in range(N_CORES):
        xT, xp = _host_prep(x[ci * B_LOC:(ci + 1) * B_LOC])
        in_maps.append({
            "xT": xT,
            "xp": xp,
            "WT": wt_h,
            "b": bb,
            "CT": ct_h,
        })
    res = run_bass_kernel_spmd(nc, in_maps, core_ids=list(range(N_CORES)))
    LAST_RESULT = res
    out = np.empty((B, S, C, H), dtype=np.float32)
    for ci in range(N_CORES):
        out[ci * B_LOC:(ci + 1) * B_LOC] = _host_decode(res.results[ci]["out"])
    return out
